# revision 21
# baseline (speedup 1.0000x reference)
"""TGCN (dense-graph GRU) Trainium2 kernel, 8-core SPMD, no collectives.

Math (per reference):
  xh_t = relu(x_t @ fc_w + fc_b)                    [N, H]
  S_t  = adj @ xh_t                                 (assoc: adj@(xh@W) = (adj@xh)@W)
  z_t  = sigmoid(S_t @ Mz + h @ Lz_bot + bz)        Mz = Wz @ Lz_top (host-folded)
  r_t  = sigmoid(S_t @ Mr + h @ Lr_bot + br)
  ht_t = tanh   (S_t @ Mh + (h*r) @ Lh_bot + bh)
  h    = z*h + (1-z)*ht

Sharding: row-partition adj across 8 cores (512 nodes each); the GRU cell is
row-local so each core runs the whole time loop independently. x replicated.

Over the 361-435us bf16 baseline (same-session A/B: 435 -> 370us traced):
- S matmul in fp8e4 + DoubleRow perf mode: 16 matmuls/pair, each
  contracting 2x128 nodes ([Ki,2,M] lhsT / [Ki,2,N] rhs 3D APs). adj is
  host-scaled by 4096 into e4m3 range (the 1/4096 folds back into the
  gate weights); the relu writes xh directly as fp8. Quantization noise
  averages out over the 4096-deep nonnegative contraction (measured rel
  err 4.2e-3 vs 4.1e-3 for all-bf16). The DR burst runs ~2x faster per
  k-tile than bf16, taking the S matmul from 8.3us to ~1.7us per pair.
- h-candidate matmul split K=64+K=64: the S-half issues right after the
  S copies (off the sequential GRU chain); only the (h*r)-half sits on
  the chain. Removes the ChS concat tiles and 2 PSUM-drain copies/pair.
- Off-chain cell ops (1-z, z*h) on GpSimd (Pool). Pool has no PSUM port,
  so relus/S-copies/sigma/tanh stay on ACT/DVE, balanced ~50/50.
- xh PSUM pool widened to 5 banks so the next pair's xh matmuls never
  wait on this pair's relus (the strict ACT/DVE FIFOs run relus late,
  behind stalled chain ops; slack beats reordering).
- x stream: 4-chunk DMA per pair on 2 queues, prefetched 2 pairs ahead.
- Emission interleave places xh groups / halves of the S burst in front
  of every chain-dependent matmul so the in-order PE queue has work
  while sigma/HR/tanh complete. Hardware-probed constraints that shaped
  this: elementwise ops need all INPUT partition bases equal (output
  base may shift), matmul operands must start at partition 0, and the
  48-step sequential cell chain (~4.3us/step) is the binding floor.

Layout: feature-major on chip; time steps processed in pairs (adj matmul
M=128 = 2 steps x 64 feats). Gate rhs is a [S_t.T; h.T] concat tile whose
bottom half IS the recurrent state, rotating over 4 buffers.
"""

import os
import sys

sys.path.insert(0, "/opt/trn_rl_repo")

import numpy as np
import ml_dtypes

T, N, F_IN, H1, F_OUT = 48, 4096, 64, 64, 64
NCORES = 8
NS = N // NCORES          # nodes per core = 512
PAIRS = T // 2            # 24
KT = N // 128             # 32 contraction tiles for the adj matmul
KP = KT // 2              # 16 DoubleRow k-pairs
ADJ_SCALE = 4096.0        # adj * 4096 fits e4m3; folded out of gate weights

_cache = {}


def _build():
    import concourse.bass as bass
    import concourse.mybir as mybir
    import concourse.tile as tile
    from concourse import bacc

    f32 = mybir.dt.float32
    bf16 = mybir.dt.bfloat16
    fp8 = mybir.dt.float8e4
    AF = mybir.ActivationFunctionType
    DR = mybir.MatmulPerfMode.DoubleRow

    nc = bacc.Bacc(
        "TRN2",
        target_bir_lowering=False,
        debug=False,
        enable_asserts=False,
        num_devices=NCORES,
    )

    # DRAM parameters (per-core shapes)
    adjT_d = nc.dram_tensor("adjT", [128, KT, NS], fp8, kind="ExternalInput").ap()
    xT_d = nc.dram_tensor("xT", [PAIRS, F_IN, 2, N], bf16, kind="ExternalInput").ap()
    fcw_d = nc.dram_tensor("fcw", [F_IN, H1], bf16, kind="ExternalInput").ap()
    wzr_d = nc.dram_tensor("wzr", [128, 128], bf16, kind="ExternalInput").ap()
    mh_d = nc.dram_tensor("mh", [F_OUT, F_OUT], bf16, kind="ExternalInput").ap()
    lh_d = nc.dram_tensor("lh", [F_OUT, F_OUT], bf16, kind="ExternalInput").ap()
    bz_d = nc.dram_tensor("bz", [F_OUT, 1], f32, kind="ExternalInput").ap()
    br_d = nc.dram_tensor("br", [F_OUT, 1], f32, kind="ExternalInput").ap()
    bh_d = nc.dram_tensor("bh", [F_OUT, 1], f32, kind="ExternalInput").ap()
    out_d = nc.dram_tensor("out", [F_OUT, NS], f32, kind="ExternalOutput").ap()

    with tile.TileContext(nc) as tc:
        with (
            tc.tile_pool(name="const", bufs=1) as constp,
            tc.tile_pool(name="state", bufs=1) as statep,
            tc.tile_pool(name="xt", bufs=3) as xtp,
            tc.tile_pool(name="xh", bufs=3) as xhp,
            tc.tile_pool(name="gw", bufs=3) as gwp,
            tc.tile_pool(name="psx", bufs=2, space="PSUM") as psxp,
            tc.tile_pool(name="pss", bufs=2, space="PSUM") as pssp,
            tc.tile_pool(name="pszr", bufs=1, space="PSUM") as pszrp,
            tc.tile_pool(name="psh", bufs=1, space="PSUM") as pshp,
        ):
            # ---- constants ----
            fcw_sb = constp.tile([F_IN, H1], bf16)
            nc.sync.dma_start(out=fcw_sb[:], in_=fcw_d[:])
            adjT_sb = constp.tile([128, KT, NS], fp8)
            for q, eng in enumerate((nc.sync, nc.gpsimd, nc.gpsimd, nc.gpsimd)):
                eng.dma_start(
                    out=adjT_sb[:, q * 8 : (q + 1) * 8, :],
                    in_=adjT_d[:, q * 8 : (q + 1) * 8, :],
                )
            wzr_sb = constp.tile([128, 128], bf16)
            mh_sb = constp.tile([F_OUT, F_OUT], bf16)
            lh_sb = constp.tile([F_OUT, F_OUT], bf16)
            bz_sb = constp.tile([F_OUT, 1], f32)
            br_sb = constp.tile([F_OUT, 1], f32)
            bh_sb = constp.tile([F_OUT, 1], f32)
            for dst, src in (
                (wzr_sb, wzr_d), (mh_sb, mh_d), (lh_sb, lh_d),
                (bz_sb, bz_d), (br_sb, br_d), (bh_sb, bh_d),
            ):
                nc.gpsimd.dma_start(out=dst[:], in_=src[:])

            # ---- state ----
            # Concat rhs tiles for the K=128 zr matmul: rows 0-63 = S_t.T
            # (refreshed per step), rows 64-127 = h.T. The combine writes h
            # into the next buffer of the 4-rotation.
            CzS = []
            for i in range(4):
                czsi = statep.tile([128, NS], bf16, tag=f"CzS{i}")
                CzS.append(czsi)
            nc.vector.memset(CzS[0][:], 0.0)

            def emit_w_mms(xt, xh, w):
                # xh-pair matmuls: out[128 nodes, 64] = xT_slice.T @ fcw.
                # 16 matmuls fill a double-bank PSUM tile; the relu drain is
                # emitted separately (emit_w_relu) so it can be slotted into
                # a chain-idle window of the ACT/DVE FIFOs.
                ps = psxp.tile([128, 1024], mybir.dt.float32)
                for j in range(8):
                    k = 8 * w + j
                    for s in (0, 1):
                        nc.tensor.matmul(
                            ps[:, j * 128 + s * 64 : j * 128 + (s + 1) * 64],
                            lhsT=xt[:, s, k * 128 : (k + 1) * 128],
                            rhs=fcw_sb[:],
                            start=True, stop=True,
                        )
                return ps

            def emit_w_relu(ps, xh, w, act_relu):
                xh_out = xh[:, 8 * w : 8 * (w + 1), :].rearrange(
                    "p a b -> p (a b)"
                )
                if act_relu:
                    nc.scalar.activation(xh_out, ps[:], AF.Relu)
                else:
                    nc.vector.tensor_relu(xh_out, ps[:])

            HNS = NS // 2

            def emit_gru_front(step):
                # Wavefront halves: every chain op is split into L (cols
                # 0:256) and R (cols 256:512) so the left wave advances at
                # roughly half the per-op latency, with the right wave
                # trailing in the engine gaps. zr + S-half of the h matmul +
                # sigmoids + products.
                cur = CzS[step % 4]
                H = cur[64:128, :]
                L, Rr = slice(0, HNS), slice(HNS, NS)

                ps_zr = pszrp.tile([128, NS], mybir.dt.float32, tag="ps_zr")
                nc.tensor.matmul(ps_zr[:, L], lhsT=wzr_sb[:], rhs=cur[:, L],
                                 start=True, stop=True)
                nc.tensor.matmul(ps_zr[:, Rr], lhsT=wzr_sb[:], rhs=cur[:, Rr],
                                 start=True, stop=True)
                # S-half of the h-candidate: depends only on the S copy, so
                # it runs off the sequential chain (start of an accum group
                # finished later by the (h*r)-half in emit_gru_back).
                # L-half only here: the R-half S-matmul is emitted in
                # emit_gru_back after h2L closes the L accumulation group -
                # concurrently-open groups on one PSUM tile corrupt the
                # earlier group's partial sums.
                ps_h = pshp.tile([F_OUT, NS], mybir.dt.float32, tag="ps_h")
                nc.tensor.matmul(ps_h[:, L], lhsT=mh_sb[:], rhs=cur[0:64, L],
                                 start=True, stop=False)
                # sigRL first (heads the L wave), then the whole-width Z (it
                # feeds only the off-chain ZC/A1), then sigRR.
                R = gwp.tile([128, NS], bf16, tag="R")
                nc.scalar.activation(R[64:128, L], ps_zr[64:128, L],
                                     AF.Sigmoid, bias=br_sb[:])
                Z = gwp.tile([128, NS], bf16, tag="Z")
                nc.scalar.activation(Z[64:128, :], ps_zr[0:64, :],
                                     AF.Sigmoid, bias=bz_sb[:])
                nc.scalar.activation(R[64:128, Rr], ps_zr[64:128, Rr],
                                     AF.Sigmoid, bias=br_sb[:])
                HR = gwp.tile([F_OUT, NS], bf16, tag="HR")
                nc.vector.tensor_mul(HR[:, L], H[:, L], R[64:128, L])
                A1 = gwp.tile([128, NS], bf16, tag="A1")
                nc.vector.tensor_mul(A1[64:128, :], Z[64:128, :], H)
                nc.vector.tensor_mul(HR[:, Rr], H[:, Rr], R[64:128, Rr])
                # off-chain: 1-z on Pool (kept whole-width)
                ZC = gwp.tile([128, NS], bf16, tag="ZC")
                nc.gpsimd.tensor_scalar(ZC[64:128, :], Z[64:128, :], -1.0, 1.0,
                                        mybir.AluOpType.mult, mybir.AluOpType.add)
                return ps_h, HR, ZC, A1

            def emit_gru_back(step, ps_h, HR, ZC, A1):
                # (h*r)-half of the h matmul + tanh + combine, wavefronted;
                # the L-wave combine (b1L, adL) is emitted before the R-wave
                # tanh consumers so adL never queues behind tanhR.
                nxt = CzS[(step + 1) % 4]
                cur = CzS[step % 4]
                L, Rr = slice(0, HNS), slice(HNS, NS)
                nc.tensor.matmul(ps_h[:, L], lhsT=lh_sb[:], rhs=HR[:, L],
                                 start=False, stop=True)
                nc.tensor.matmul(ps_h[:, Rr], lhsT=mh_sb[:], rhs=cur[0:64, Rr],
                                 start=True, stop=False)
                nc.tensor.matmul(ps_h[:, Rr], lhsT=lh_sb[:], rhs=HR[:, Rr],
                                 start=False, stop=True)
                HT = gwp.tile([128, NS], bf16, tag="HT")
                nc.scalar.activation(HT[64:128, L], ps_h[:, L], AF.Tanh,
                                     bias=bh_sb[:])
                nc.scalar.activation(HT[64:128, Rr], ps_h[:, Rr], AF.Tanh,
                                     bias=bh_sb[:])
                B1 = gwp.tile([128, NS], bf16, tag="B1")
                nc.vector.tensor_mul(B1[64:128, L], ZC[64:128, L],
                                     HT[64:128, L])
                nc.vector.tensor_add(nxt[64:128, L], A1[64:128, L],
                                     B1[64:128, L])
                nc.vector.tensor_mul(B1[64:128, Rr], ZC[64:128, Rr],
                                     HT[64:128, Rr])
                nc.vector.tensor_add(nxt[64:128, Rr], A1[64:128, Rr],
                                     B1[64:128, Rr])

            def emit_xt_dma(p, xt):
                # 4-chunk split, all on the sync queue: a DMA config costs
                # ~670ns of engine time, and on gpsimd it was delaying ZC
                # (whose lateness stalls the combine's b1).
                for c in range(4):
                    nc.sync.dma_start(
                        out=xt[:, :, c * 1024 : (c + 1) * 1024],
                        in_=xT_d[p][:, :, c * 1024 : (c + 1) * 1024],
                    )

            # ---- main loop, software-pipelined one pair deeper than before:
            # iteration `it` runs the S burst for pair `it` using xh[it]
            # computed an entire iteration ago (so it never waits on this
            # iteration's relus and starts the instant the PE frees up),
            # the gates for pair it-1, and the xh production for pair it+1.
            # S-chunks and xh groups are spread between chain ops so the
            # in-order PE queue holds ~1-2us of ready filler at every chain
            # dependency point - bridging the chain latency keeps the PE
            # pstate high, which is worth ~2x on the fp8 burst itself. ----
            def emit_s_chunk(psS, xh, lo, hi):
                for kp in range(lo, hi):
                    nc.tensor.matmul(
                        psS[:],
                        lhsT=xh[:, 2 * kp : 2 * kp + 2, :],
                        rhs=adjT_sb[:, 2 * kp : 2 * kp + 2, :],
                        start=(kp == 0), stop=(kp == KP - 1),
                        perf_mode=DR,
                    )

            xt_t0 = xtp.tile([F_IN, 2, N], bf16, tag="xt")
            emit_xt_dma(0, xt_t0)
            xt_t1 = xtp.tile([F_IN, 2, N], bf16, tag="xt")
            emit_xt_dma(1, xt_t1)
            xt_q = [xt_t0, xt_t1]
            # prologue: xh[0] in full (PE-only work, nothing to overlap yet)
            xh_cur = xhp.tile([128, KT, 128], fp8)
            for w in range(4):
                ps = emit_w_mms(xt_q[0], xh_cur, w)
                emit_w_relu(ps, xh_cur, w, act_relu=(w % 2 == 1))
            xt_q.pop(0)

            # Steady-state schedule, provisioned for FULL-pstate PE rates so
            # the queue never runs dry (an idle PE drops to the 1.2GHz
            # pstate and everything runs 2x slow until 3us of continuous
            # busy re-ramps it). Filler sized to the chain-dependency gaps:
            #   zr(t0)-shadow  (~1.3us): 3 S chunks + W01 mms
            #   h2(t0)-shadow  (~1.8us): 7 S chunks + W23 mms
            #   zr(t1)-shadow  (~1.3us): 3 S chunks + W45 mms
            #   h2(t1)-shadow  (~1.8us): 3 S chunks + W67 mms
            # Elementwise drains in the measured chain-idle FIFO windows:
            #   ACT: W01 relu after tanh(t0); both S copies after tanh(t1)
            #   DVE: W23 relu after ad(t0); W45 after ad(t1); W67 at end
            for it in range(PAIRS + 1):
                sp, gp, xp = it, it - 1, it + 1
                do_s = sp < PAIRS
                do_g = gp >= 0
                do_x = xp < PAIRS
                psw = [None] * 4
                if do_s:
                    psS = pssp.tile([128, NS], mybir.dt.float32)
                if do_x:
                    xt = xt_q.pop(0)
                    xh_nxt = xhp.tile([128, KT, 128], fp8)
                if do_g:
                    fr0 = emit_gru_front(2 * gp)
                if do_s:
                    emit_s_chunk(psS, xh_cur, 0, 2)
                if do_x:
                    psw[0] = emit_w_mms(xt, xh_nxt, 0)
                if do_g:
                    emit_gru_back(2 * gp, *fr0)
                if do_x:
                    emit_w_relu(psw[0], xh_nxt, 0, act_relu=True)
                if do_s:
                    emit_s_chunk(psS, xh_cur, 2, 5)
                if do_x:
                    psw[1] = emit_w_mms(xt, xh_nxt, 1)
                if do_g:
                    fr1 = emit_gru_front(2 * gp + 1)
                if do_s:
                    emit_s_chunk(psS, xh_cur, 5, 7)
                if do_x:
                    psw[2] = emit_w_mms(xt, xh_nxt, 2)
                if do_g:
                    emit_gru_back(2 * gp + 1, *fr1)
                if do_s:
                    emit_s_chunk(psS, xh_cur, 7, 11)
                if do_x:
                    emit_w_relu(psw[1], xh_nxt, 1, act_relu=False)
                    psw[3] = emit_w_mms(xt, xh_nxt, 3)
                if do_s:
                    # refresh concat tops for pair sp's two steps (consumed
                    # by the gates of the next iteration); both in the ACT
                    # window after tanh(t1)
                    emit_s_chunk(psS, xh_cur, 11, KP)
                    s0, s1 = (2 * sp) % 4, (2 * sp + 1) % 4
                    nc.scalar.copy(CzS[s0][0:64, :], psS[0:64, :])
                    nc.scalar.copy(CzS[s1][0:64, :], psS[64:128, :])
                if do_x:
                    emit_w_relu(psw[2], xh_nxt, 2, act_relu=False)
                    emit_w_relu(psw[3], xh_nxt, 3, act_relu=False)
                    xh_cur = xh_nxt
                    if xp + 1 < PAIRS:
                        xt_n = xtp.tile([F_IN, 2, N], bf16, tag="xt")
                        emit_xt_dma(xp + 1, xt_n)
                        xt_q.append(xt_n)

            Hout = statep.tile([F_OUT, NS], f32)
            nc.scalar.copy(Hout[:], CzS[(2 * PAIRS) % 4][64:128, :])
            nc.sync.dma_start(out=out_d[:], in_=Hout[:])

    nc.compile()
    return nc


def _prep_inputs(x, adj, fc_w, Wz, Wr, Wh, Lz, Lr, Lh, bz, br, bh):
    bf16 = ml_dtypes.bfloat16
    fp8 = ml_dtypes.float8_e4m3
    f32 = np.float32

    # x [T, N, F] -> [PAIRS, F, step, N] (features on partitions)
    xT = np.ascontiguousarray(
        x.reshape(PAIRS, 2, N, F_IN).transpose(0, 3, 1, 2)
    ).astype(bf16)
    fcw = fc_w.astype(bf16)

    def fold(W, L):
        # W @ L_top, with the 1/ADJ_SCALE of the fp8 adj folded in
        return (
            (W.astype(np.float64) @ L[:F_OUT].astype(np.float64)) / ADJ_SCALE
        ).astype(bf16)

    mz, mr, mh = fold(Wz, Lz), fold(Wr, Lr), fold(Wh, Lh)
    mzr = np.concatenate([mz, mr], axis=1)  # [64, 128]: z cols | r cols
    lzr = np.concatenate(
        [Lz[F_OUT:].astype(bf16), Lr[F_OUT:].astype(bf16)], axis=1
    )
    wzr = np.concatenate([mzr, lzr], axis=0)  # [128, 128]
    shared = {
        "xT": xT, "fcw": fcw, "wzr": wzr,
        "mh": mh, "lh": Lh[F_OUT:].astype(bf16),
        "bz": bz.reshape(F_OUT, 1).astype(f32),
        "br": br.reshape(F_OUT, 1).astype(f32),
        "bh": bh.reshape(F_OUT, 1).astype(f32),
    }
    in_maps = []
    for c in range(NCORES):
        m = dict(shared)
        at = adj[c * NS : (c + 1) * NS, :].T * ADJ_SCALE  # [N, NS] in [0,1]
        m["adjT"] = np.ascontiguousarray(
            at.reshape(KT, 128, NS).transpose(1, 0, 2)
        ).astype(fp8)
        in_maps.append(m)
    return in_maps


def kernel(x, adj, fc_w, fc_b, Wz, Wr, Wh, Lz, Lr, Lh, bz, br, bh):
    x = np.asarray(x, np.float32)
    adj = np.asarray(adj, np.float32)
    args = [np.asarray(a, np.float32) for a in (fc_w, Wz, Wr, Wh, Lz, Lr, Lh, bz, br, bh)]
    fc_b = np.asarray(fc_b, np.float32)
    if np.any(fc_b != 0.0):
        # fc_b can't fold into the per-partition activation bias (it varies
        # along the free dim); the reference always passes zeros.
        return _numpy_ref(x, adj, args[0], fc_b, *args[1:])

    from concourse.bass_utils import run_bass_kernel_spmd

    if "nc" not in _cache:
        _cache["nc"] = _build()
    nc = _cache["nc"]

    in_maps = _prep_inputs(x, adj, *args)
    trace = bool(int(os.environ.get("BASS_KERNEL_TRACE", "0")))
    kwargs = {}
    if trace:
        _install_trace_shim()
        tmpdir = os.environ.get("BASS_KERNEL_TRACE_DIR")
        if tmpdir:
            os.makedirs(tmpdir, exist_ok=True)
            kwargs["tmpdir"] = tmpdir
    res = run_bass_kernel_spmd(
        nc, in_maps, core_ids=list(range(NCORES)), trace=trace, **kwargs
    )
    _cache["last_result"] = res

    out = np.empty((1, N, F_OUT), np.float32)
    for c in range(NCORES):
        out[0, c * NS : (c + 1) * NS, :] = res.results[c]["out"].T
    return out


def _install_trace_shim():
    """Register the NTFF profile hook (this image's antenv lacks axon_hooks)
    and stub out the artifact upload so profiling works offline."""
    import types

    try:
        from antenv import axon_hooks  # noqa: F401
        return
    except ImportError:
        pass
    sys.path.insert(0, "/root/.axon_site")
    from trn_agent_boot.trn_boot import _ntff_profile_via_ctypes

    hook = _ntff_profile_via_ctypes("/opt/axon/libaxon_pjrt.so")
    m = types.ModuleType("antenv.axon_hooks")
    m.get_axon_ntff_profile_hook = lambda: hook
    m.set_axon_ntff_profile_hook = lambda h: None
    sys.modules["antenv.axon_hooks"] = m
    import antenv

    antenv.axon_hooks = m
    from concourse import bass_utils as _bu

    _bu.upload_artifacts = lambda tmpdir: tmpdir


def _numpy_ref(x, adj, fc_w, fc_b, Wz, Wr, Wh, Lz, Lr, Lh, bz, br, bh):
    def sigmoid(v):
        return 1.0 / (1.0 + np.exp(-v))

    xh = np.maximum(x @ fc_w + fc_b, 0.0)
    h = np.zeros((N, F_OUT), np.float32)
    for t in range(T):
        s = adj @ xh[t]
        az, ar, ah = s @ Wz, s @ Wr, s @ Wh
        z = sigmoid(np.concatenate([az, h], -1) @ Lz + bz)
        r = sigmoid(np.concatenate([ar, h], -1) @ Lr + br)
        ht = np.tanh(np.concatenate([ah, h * r], -1) @ Lh + bh)
        h = z * h + (1.0 - z) * ht
    return h[None].astype(np.float32)

